# revision 23
# baseline (speedup 1.0000x reference)
"""CollaborativeAttention Trainium2 kernel (v2, all-fp8 DoubleRow).

Sharding: 8 cores = (batch b in {0,1}) x (512-query-row block). Each core
computes its 512 output rows end to end; k/v/content-bias are computed
redundantly within each batch group, so no device collectives are needed.
Each core's own query rows are permuted to j-columns 0:512 on the host, so
the SPMD program always projects q from quarter 0 (softmax is j-order
invariant as long as k/v/cb share the permutation, which they do).

Precision: the attention contribution to the output is tiny relative to the
residual (std ~0.007 vs 1.0), so every matmul runs in fp8e4m3 DoubleRow
(2x PE throughput) with scales folded on the host:
  x -> fp8 directly; Wq,Wk,Wv,Wd -> fp8 x16 (keeps them out of the fp8
  subnormal range); Wcb -> fp8 x2; mixing -> f32 /2.
  scores_psum = (8 q.m)(16 k) = 1024*(q.m.k/8)  -> exp scale 1/1024
  cb_psum = 2*x.Wcb -> cbT = psum/16 = cb/8 (the exp bias)
  ctx_psum/den = 16*ctx -> fp8 ctxn; dense_psum = 256*(ctx@Wd)
  residual handled at 256x: xr2 = 256*(x + bd + Wd@bv); layernorm is
  scale-invariant given eps' = 256^2 * 1e-5.

Per-core DR matmul layout convention: contraction index c = g*256 + ko*128 + p
with p the partition, stationary [128, 2(ko), cols], moving [128, 2(ko), free].

Dataflow per quarter (j-block of 512 keys):
  kT8[c,j] (fp8, 32 MMs) ; vA[j,(h,65)] + cb (fp8, 48 MMs) ;
  for head-groups of 4: scores (4 banks) -> exp(+cb bias) -> pr fp8 ->
  ctx DR into [65,512] banks (row 64 = ones column = softmax denominator),
  drained into SBUF accumulator ctxu[65,16,512].
Tail: den -> DRAM -> reciprocal -> broadcast; ctxn fp8 = ctxu*rec;
  dense DR + residual + LN.
"""

import sys

if '/opt/trn_rl_repo' not in sys.path:
    sys.path.insert(0, '/opt/trn_rl_repo')

import numpy as np

_CACHE = {}

B, S, D, H = 2, 2048, 1024, 16
R = 512          # query rows per core
NQ = 4           # j quarters


def _build():
    import concourse.bass as bass
    from concourse import bacc
    import concourse.mybir as mybir
    import concourse.tile as tile

    f32 = mybir.dt.float32
    bf16 = mybir.dt.bfloat16
    f8 = mybir.dt.float8e4
    AF = mybir.ActivationFunctionType
    ALU = mybir.AluOpType
    DR = mybir.MatmulPerfMode.DoubleRow

    nc = bacc.Bacc("TRN2", debug=False, target_bir_lowering=False)

    xt8_d = nc.dram_tensor("xt8", [128, 4, 2, S], f8, kind="ExternalInput").ap()
    wq8_d = nc.dram_tensor("wq8", [128, 4, 2, D], f8, kind="ExternalInput").ap()
    wk8_d = nc.dram_tensor("wk8", [128, 4, 2, D], f8, kind="ExternalInput").ap()
    wv8_d = nc.dram_tensor("wv8", [128, 4, 2, D], f8, kind="ExternalInput").ap()
    wd8_d = nc.dram_tensor("wd8", [64, 8, 2, D], f8, kind="ExternalInput").ap()
    wcb8_d = nc.dram_tensor("wcb8", [128, 4, 2, H], f8, kind="ExternalInput").ap()
    mt_d = nc.dram_tensor("mt", [128, 8, H], f32, kind="ExternalInput").ap()
    xr2_d = nc.dram_tensor("xr2", [R, D], f32, kind="ExternalInput").ap()
    gam_d = nc.dram_tensor("gamma2", [1, D], bf16, kind="ExternalInput").ap()
    bet_d = nc.dram_tensor("beta2", [1, D], bf16, kind="ExternalInput").ap()
    out_d = nc.dram_tensor("out", [R, D], f32, kind="ExternalOutput").ap()

    def bcast_row(ap_row, n):
        return bass.AP(tensor=ap_row.tensor, offset=ap_row.offset,
                       ap=[[0, 128], [1, n]])

    with tile.TileContext(nc) as tc:
        with tc.tile_pool(name="sp", bufs=1) as sp, \
             tc.tile_pool(name="pp", bufs=1, space="PSUM") as pp, \
             tc.tile_pool(name="dp", bufs=1, space="DRAM") as dp:

            den_dram = dp.tile([1, H, R], bf16, tag="dend")
            rec_dram = dp.tile([H, R], f32, tag="recd")

            # resident inputs (xt8 split so quarter 0 lands first)
            xt8 = sp.tile([128, 4, 2, S], f8, tag="xt8")
            nc.sync.dma_start(out=xt8[:, :, :, 0:R], in_=xt8_d[:, :, :, 0:R])
            wq8 = sp.tile([128, 4, 2, D], f8, tag="wq8")
            for hh2 in range(2):
                nc.sync.dma_start(out=wq8[:, :, :, hh2 * 512:(hh2 + 1) * 512],
                                  in_=wq8_d[:, :, :, hh2 * 512:(hh2 + 1) * 512])
            wk8 = sp.tile([128, 4, 2, D], f8, tag="wk8")
            for hh2 in range(2):
                nc.sync.dma_start(out=wk8[:, :, :, hh2 * 512:(hh2 + 1) * 512],
                                  in_=wk8_d[:, :, :, hh2 * 512:(hh2 + 1) * 512])
            for qq in range(1, 4):
                nc.sync.dma_start(out=xt8[:, :, :, qq * R:(qq + 1) * R],
                                  in_=xt8_d[:, :, :, qq * R:(qq + 1) * R])
            wv8 = sp.tile([128, 4, 2, D], f8, tag="wv8")
            nc.sync.dma_start(out=wv8, in_=wv8_d)
            wd8 = sp.tile([64, 8, 2, D], f8, tag="wd8")
            nc.sync.dma_start(out=wd8, in_=wd8_d)
            wcb8 = sp.tile([128, 4, 2, H], f8, tag="wcb8")
            nc.sync.dma_start(out=wcb8, in_=wcb8_d)
            mt_sb = sp.tile([128, 8, H], f32, tag="mt")
            nc.sync.dma_start(out=mt_sb, in_=mt_d)
            gamB = sp.tile([128, D], bf16, tag="gamB")
            nc.sync.dma_start(out=gamB, in_=bcast_row(gam_d[0:1, :], D))
            betB = sp.tile([128, D], bf16, tag="betB")
            nc.sync.dma_start(out=betB, in_=bcast_row(bet_d[0:1, :], D))
            epsT = sp.tile([128, 1], f32, tag="epsT")
            nc.vector.memset(epsT, 1e-5 * 256.0 * 256.0)

            # persistent intermediates
            qT = sp.tile([128, 8, R], bf16, tag="qT")
            qm = sp.tile([128, H, 4, 2, R], f8, tag="qm")
            ctxu = sp.tile([65, H, R], bf16, tag="ctxu")
            ctxn = sp.tile([64, 8, 2, R], f8, tag="ctxn")

            # q projection (own rows = j 0:512), psum = 16*q
            for cb2 in range(8):
                ps = pp.tile([128, R], f32, tag="A", bufs=5, name=f"qps_{cb2}")
                for dg in range(4):
                    nc.tensor.matmul(ps, wq8[:, dg, :, cb2 * 128:(cb2 + 1) * 128],
                                     xt8[:, dg, :, 0:R],
                                     start=(dg == 0), stop=(dg == 3),
                                     perf_mode=DR)
                nc.scalar.copy(out=qT[:, cb2, :], in_=ps)

            # qm for head-groups 0,1 on the scalar engine (early window)
            for h in range(8):
                for g in range(4):
                    for ko in range(2):
                        nc.scalar.mul(out=qm[:, h, g, ko, :],
                                      in_=qT[:, 2 * g + ko, :],
                                      mul=mt_sb[:, 2 * g + ko, h:h + 1])

            for q in range(NQ):
                jq = slice(q * R, (q + 1) * R)

                # k projection for this quarter -> fp8 (16*k)
                kT8 = sp.tile([128, 4, 2, R], f8, tag="kT8", bufs=2,
                              name=f"kT8_{q}")
                for cb2 in range(8):
                    ps = pp.tile([128, R], f32, tag="A", bufs=5,
                                 name=f"kps_{q}_{cb2}")
                    for dg in range(4):
                        nc.tensor.matmul(ps, wk8[:, dg, :, cb2 * 128:(cb2 + 1) * 128],
                                         xt8[:, dg, :, jq],
                                         start=(dg == 0), stop=(dg == 3),
                                         perf_mode=DR)
                    nc.vector.tensor_copy(out=kT8[:, cb2 // 2, cb2 % 2, :], in_=ps)

                # v projection (+ ones column) and content bias
                vA = sp.tile([128, 2, 2, H * 65], f8, tag="vA", bufs=2,
                             name=f"vA_{q}")
                cbT = sp.tile([128, 4, H], f32, tag="cbT", bufs=2, name=f"cbT_{q}")
                for jtp in range(2):
                    for ko in range(2):
                        ev = vA[:, jtp, ko, :].rearrange("p (h u) -> p h u", u=65)
                        nc.vector.memset(ev[:, :, 64:65], 1.0)
                for jt in range(4):
                    jb = slice(q * R + jt * 128, q * R + jt * 128 + 128)
                    psv = [pp.tile([128, R], f32, tag="A", bufs=5,
                                   name=f"vps_{q}_{jt}_{half}")
                           for half in range(2)]
                    pscb = pp.tile([128, H], f32, tag="C", bufs=3,
                                   name=f"cbps_{q}_{jt}")
                    for dg in range(4):
                        for half in range(2):
                            nc.tensor.matmul(psv[half], xt8[:, dg, :, jb],
                                             wv8[:, dg, :, half * 512:(half + 1) * 512],
                                             start=(dg == 0), stop=(dg == 3),
                                             perf_mode=DR)
                        nc.tensor.matmul(pscb, xt8[:, dg, :, jb], wcb8[:, dg, :, :],
                                         start=(dg == 0), stop=(dg == 3),
                                         perf_mode=DR)
                    ev = vA[:, jt // 2, jt % 2, :].rearrange("p (h u) -> p h u", u=65)
                    for half in range(2):
                        nc.vector.tensor_copy(
                            out=ev[:, half * 8:(half + 1) * 8, 0:64],
                            in_=psv[half].rearrange("p (h u) -> p h u", u=64))
                    nc.vector.tensor_scalar(out=cbT[:, jt, :], in0=pscb,
                                            scalar1=1.0 / 16.0, scalar2=None,
                                            op0=ALU.mult)

                if q == 0:
                    # remaining qm (DVE), after quarter-0 casts in program order
                    for h in range(8, H):
                        for g in range(4):
                            for ko in range(2):
                                nc.vector.tensor_scalar_mul(
                                    qm[:, h, g, ko, :], qT[:, 2 * g + ko, :],
                                    mt_sb[:, 2 * g + ko, h:h + 1])

                # attention: head groups of 4
                for hg in range(4):
                    prs = []
                    for hh in range(4):
                        pr = sp.tile([128, 2, 2, R], f8, tag="pr", bufs=4,
                                     name=f"pr_{q}_{hg}_{hh}")
                        prs.append(pr)
                    for jt in range(4):
                        pss = []
                        for hh in range(4):
                            pss.append(pp.tile([128, R], f32, tag="A", bufs=5,
                                               name=f"sps_{q}_{hg}_{jt}_{hh}"))
                        for g in range(4):
                            for hh in range(4):
                                h = hg * 4 + hh
                                nc.tensor.matmul(
                                    pss[hh], kT8[:, g, :, jt * 128:(jt + 1) * 128],
                                    qm[:, h, g, :, :],
                                    start=(g == 0), stop=(g == 3), perf_mode=DR)
                        for hh in range(4):
                            h = hg * 4 + hh
                            nc.scalar.activation(
                                out=prs[hh][:, jt // 2, jt % 2, :], in_=pss[hh],
                                func=AF.Exp, bias=cbT[:, jt, h:h + 1],
                                scale=1.0 / 1024.0)
                    for hh in range(4):
                        h = hg * 4 + hh
                        cps = pp.tile([65, R], f32, tag="C", bufs=3,
                                      name=f"cps_{q}_{hg}_{hh}")
                        for jtp in range(2):
                            nc.tensor.matmul(cps, vA[:, jtp, :, h * 65:h * 65 + 65],
                                             prs[hh][:, jtp, :, :],
                                             start=(jtp == 0), stop=(jtp == 1),
                                             perf_mode=DR)
                        if q == 0:
                            nc.vector.tensor_copy(out=ctxu[:, h, :], in_=cps)
                        else:
                            nc.vector.tensor_tensor(out=ctxu[:, h, :], in0=cps,
                                                    in1=ctxu[:, h, :], op=ALU.add)

                    if q == NQ - 1:
                        # den -> reciprocal -> normalized fp8 ctx, per head
                        # group, overlapping the remaining score matmuls
                        h0 = hg * 4
                        nc.sync.dma_start(out=den_dram[:, h0:h0 + 4, :],
                                          in_=ctxu[64:65, h0:h0 + 4, :])
                        dl = sp.tile([4, R], bf16, tag="dl", bufs=2,
                                     name=f"dl_{hg}")
                        dsrc = bass.AP(tensor=den_dram.tensor,
                                       offset=den_dram[0:1, h0:h0 + 4, :].offset,
                                       ap=[[R, 4], [1, R]])
                        nc.sync.dma_start(out=dl, in_=dsrc)
                        rec4 = sp.tile([4, R], f32, tag="rec", bufs=2,
                                       name=f"rec_{hg}")
                        nc.vector.reciprocal(out=rec4, in_=dl)
                        nc.sync.dma_start(out=rec_dram[h0:h0 + 4, :], in_=rec4)
                        for hh in range(4):
                            h = h0 + hh
                            rb = sp.tile([64, R], f32, tag="rb", bufs=2,
                                         name=f"rb_{h}")
                            src = bass.AP(tensor=rec_dram.tensor,
                                          offset=rec_dram[h:h + 1, :].offset,
                                          ap=[[0, 64], [1, R]])
                            nc.sync.dma_start(out=rb, in_=src)
                            nc.vector.tensor_tensor(
                                out=ctxn[:, h // 2, h % 2, :],
                                in0=ctxu[0:64, h, :], in1=rb, op=ALU.mult)

            # dense (psum = 256*(ctx@Wd)) + residual (xr2 = 256*(x+bd')) + LN
            for ic in range(4):
                res = sp.tile([128, D], f32, tag="res", bufs=1, name=f"res_{ic}")
                xrc = sp.tile([128, D], f32, tag="xrc", bufs=1, name=f"xrc_{ic}")
                nc.sync.dma_start(out=xrc, in_=xr2_d[ic * 128:(ic + 1) * 128, :])
                for oh in range(2):
                    ps = pp.tile([128, 512], f32, tag="A", bufs=5,
                                 name=f"dps_{ic}_{oh}")
                    for s in range(8):
                        nc.tensor.matmul(ps, ctxn[:, s, :, ic * 128:(ic + 1) * 128],
                                         wd8[:, s, :, oh * 512:(oh + 1) * 512],
                                         start=(s == 0), stop=(s == 7),
                                         perf_mode=DR)
                    nc.vector.tensor_tensor(
                        out=res[:, oh * 512:(oh + 1) * 512], in0=ps,
                        in1=xrc[:, oh * 512:(oh + 1) * 512], op=ALU.add)
                stats = sp.tile([128, 2, nc.vector.BN_STATS_DIM], f32, tag="stats",
                                bufs=2, name=f"stats_{ic}")
                for g in range(2):
                    nc.vector.bn_stats(out=stats[:, g, :],
                                       in_=res[:, g * 512:(g + 1) * 512])
                mv = sp.tile([128, nc.vector.BN_AGGR_DIM], f32, tag="mv", bufs=2,
                             name=f"mv_{ic}")
                nc.vector.bn_aggr(out=mv, in_=stats)
                rstd = sp.tile([128, 1], f32, tag="rstd", bufs=2, name=f"rstd_{ic}")
                nc.scalar.activation(out=rstd, in_=mv[:, 1:2], func=AF.Sqrt,
                                     bias=epsT, scale=1.0)
                nc.vector.reciprocal(out=rstd, in_=rstd)
                lnA = sp.tile([128, D], f32, tag="lnA", bufs=2, name=f"lnA_{ic}")
                nc.vector.tensor_scalar(out=lnA, in0=res, scalar1=mv[:, 0:1],
                                        scalar2=rstd, op0=ALU.subtract,
                                        op1=ALU.mult)
                nc.gpsimd.tensor_tensor(out=res, in0=lnA, in1=gamB, op=ALU.mult)
                nc.gpsimd.tensor_tensor(out=lnA, in0=res, in1=betB, op=ALU.add)
                nc.sync.dma_start(out=out_d[ic * 128:(ic + 1) * 128, :], in_=lnA)

    nc.compile()
    return nc


def _arr8(mat, scale):
    """[Drows, C] f32 -> [128, 4, 2, C] fp8 with rows d = dg*256 + ko*128 + p."""
    import ml_dtypes
    a = (mat * scale).astype(ml_dtypes.float8_e4m3)
    C = a.shape[1]
    return np.ascontiguousarray(a.reshape(4, 2, 128, C).transpose(2, 0, 1, 3))


def _prep_in_maps(inputs):
    import ml_dtypes
    f = np.float32
    x = np.ascontiguousarray(np.asarray(inputs["hidden_states"], f))
    Wq = np.asarray(inputs["Wq"], f)
    Wk = np.asarray(inputs["Wk"], f)
    Wcb = np.asarray(inputs["Wcb"], f)
    Wv = np.asarray(inputs["Wv"], f)
    bv = np.asarray(inputs["bv"], f)
    mixing = np.asarray(inputs["mixing"], f)
    Wd = np.asarray(inputs["Wd"], f)
    bd = np.asarray(inputs["bd"], f)
    gamma = np.asarray(inputs["gamma"], f)
    beta = np.asarray(inputs["beta"], f)

    bde = (bd + Wd @ bv).astype(f)
    mt = np.ascontiguousarray(
        (mixing.T * 0.5).reshape(8, 128, H).transpose(1, 0, 2)).astype(f)
    shared = {
        "wq8": _arr8(Wq.T, 16.0),
        "wk8": _arr8(Wk.T, 16.0),
        "wv8": _arr8(Wv.T, 16.0),
        # dense stationary convention: v = (2s+ko)*64 + u -> [u, s, ko, o]
        "wd8": np.ascontiguousarray(
            (Wd.T * 16.0).astype(ml_dtypes.float8_e4m3)
            .reshape(8, 2, 64, D).transpose(2, 0, 1, 3)),
        "wcb8": _arr8(Wcb.T, 2.0),
        "mt": mt,
        "gamma2": np.ascontiguousarray(gamma[None, :]).astype(ml_dtypes.bfloat16),
        "beta2": np.ascontiguousarray(beta[None, :]).astype(ml_dtypes.bfloat16),
    }
    xb8 = [np.asarray(x[b].T, ml_dtypes.float8_e4m3) for b in range(B)]
    in_maps = []
    for c in range(8):
        b, rb = divmod(c, 4)
        r0 = rb * R
        cols = np.r_[r0:r0 + R, 0:r0, r0 + R:S]
        xp = xb8[b][:, cols]
        xt8 = np.ascontiguousarray(xp.reshape(4, 2, 128, S).transpose(2, 0, 1, 3))
        xr2 = np.ascontiguousarray(256.0 * (x[b, r0:r0 + R] + bde[None, :]))
        in_maps.append({"xt8": xt8, "xr2": xr2, **shared})
    return in_maps


def _gather(results):
    out = np.empty((B, S, D), np.float32)
    for c in range(8):
        b, rb = divmod(c, 4)
        out[b, rb * R:(rb + 1) * R] = results[c]["out"]
    return out


def kernel(**inputs):
    from concourse.bass_utils import run_bass_kernel_spmd

    if "nc" not in _CACHE:
        _CACHE["nc"] = _build()
    nc = _CACHE["nc"]
    in_maps = _prep_in_maps(inputs)
    res = run_bass_kernel_spmd(nc, in_maps, core_ids=list(range(8)))
    return (_gather(res.results),)


# revision 27
# speedup vs baseline: 1.0508x; 1.0508x over previous
"""CollaborativeAttention Trainium2 kernel (v2, all-fp8 DoubleRow).

Sharding: 8 cores = (batch b in {0,1}) x (512-query-row block). Each core
computes its 512 output rows end to end; k/v/content-bias are computed
redundantly within each batch group, so no device collectives are needed.
Each core's own query rows are permuted to j-columns 0:512 on the host, so
the SPMD program always projects q from quarter 0 (softmax is j-order
invariant as long as k/v/cb share the permutation, which they do).

Precision: the attention contribution to the output is tiny relative to the
residual (std ~0.007 vs 1.0), so every matmul runs in fp8e4m3 DoubleRow
(2x PE throughput) with scales folded on the host:
  x -> fp8 directly; Wq,Wk,Wv,Wd -> fp8 x16 (keeps them out of the fp8
  subnormal range); Wcb -> fp8 x2; mixing -> f32 /2.
  scores_psum = (8 q.m)(16 k) = 1024*(q.m.k/8)  -> exp scale 1/1024
  cb_psum = 2*x.Wcb -> cbT = psum/16 = cb/8 (the exp bias)
  ctx_psum/den = 16*ctx -> fp8 ctxn; dense_psum = 256*(ctx@Wd)
  residual handled at 256x: xr2 = 256*(x + bd + Wd@bv); layernorm is
  scale-invariant given eps' = 256^2 * 1e-5.

Per-core DR matmul layout convention: contraction index c = g*256 + ko*128 + p
with p the partition, stationary [128, 2(ko), cols], moving [128, 2(ko), free].

Dataflow per quarter (j-block of 512 keys):
  kT8[c,j] (fp8, 32 MMs) ; vA[j,(h,65)] + cb (fp8, 48 MMs) ;
  for head-groups of 4: scores (4 banks) -> exp(+cb bias) -> pr fp8 ->
  ctx DR into [65,512] banks (row 64 = ones column = softmax denominator),
  drained into SBUF accumulator ctxu[65,16,512].
Tail: den -> DRAM -> reciprocal -> broadcast; ctxn fp8 = ctxu*rec;
  dense DR + residual + LN.
"""

import sys

if '/opt/trn_rl_repo' not in sys.path:
    sys.path.insert(0, '/opt/trn_rl_repo')

import numpy as np

_CACHE = {}

B, S, D, H = 2, 2048, 1024, 16
R = 512          # query rows per core
NQ = 4           # j quarters


def _build():
    import concourse.bass as bass
    from concourse import bacc
    import concourse.mybir as mybir
    import concourse.tile as tile

    f32 = mybir.dt.float32
    bf16 = mybir.dt.bfloat16
    f8 = mybir.dt.float8e4
    AF = mybir.ActivationFunctionType
    ALU = mybir.AluOpType
    DR = mybir.MatmulPerfMode.DoubleRow

    nc = bacc.Bacc("TRN2", debug=False, target_bir_lowering=False)

    xt8_d = nc.dram_tensor("xt8", [128, 4, 2, S], f8, kind="ExternalInput").ap()
    wq8_d = nc.dram_tensor("wq8", [128, 4, 2, D], f8, kind="ExternalInput").ap()
    wk8_d = nc.dram_tensor("wk8", [128, 4, 2, D], f8, kind="ExternalInput").ap()
    wv8_d = nc.dram_tensor("wv8", [128, 4, 2, D], f8, kind="ExternalInput").ap()
    wd8_d = nc.dram_tensor("wd8", [64, 8, 2, D], f8, kind="ExternalInput").ap()
    wcb8_d = nc.dram_tensor("wcb8", [128, 4, 2, H], f8, kind="ExternalInput").ap()
    mt_d = nc.dram_tensor("mt", [128, 8, H], f32, kind="ExternalInput").ap()
    xr2_d = nc.dram_tensor("xr2", [R, D], f32, kind="ExternalInput").ap()
    gam_d = nc.dram_tensor("gamma2", [1, D], bf16, kind="ExternalInput").ap()
    bet_d = nc.dram_tensor("beta2", [1, D], bf16, kind="ExternalInput").ap()
    out_d = nc.dram_tensor("out", [R, D], f32, kind="ExternalOutput").ap()

    def bcast_row(ap_row, n):
        return bass.AP(tensor=ap_row.tensor, offset=ap_row.offset,
                       ap=[[0, 128], [1, n]])

    with tile.TileContext(nc) as tc:
        with tc.tile_pool(name="sp", bufs=1) as sp, \
             tc.tile_pool(name="pp", bufs=1, space="PSUM") as pp, \
             tc.tile_pool(name="dp", bufs=1, space="DRAM") as dp:

            den_dram = dp.tile([1, H, R], bf16, tag="dend")
            rec_dram = dp.tile([H, R], f32, tag="recd")

            # resident inputs (xt8 split so quarter 0 lands first)
            xt8 = sp.tile([128, 4, 2, S], f8, tag="xt8")
            nc.sync.dma_start(out=xt8[:, :, :, 0:R], in_=xt8_d[:, :, :, 0:R])
            wq8 = sp.tile([128, 4, 2, D], f8, tag="wq8")
            for hh2 in range(2):
                nc.sync.dma_start(out=wq8[:, :, :, hh2 * 512:(hh2 + 1) * 512],
                                  in_=wq8_d[:, :, :, hh2 * 512:(hh2 + 1) * 512])
            wk8 = sp.tile([128, 4, 2, D], f8, tag="wk8")
            for hh2 in range(2):
                nc.sync.dma_start(out=wk8[:, :, :, hh2 * 512:(hh2 + 1) * 512],
                                  in_=wk8_d[:, :, :, hh2 * 512:(hh2 + 1) * 512])
            for qq in range(1, 4):
                nc.sync.dma_start(out=xt8[:, :, :, qq * R:(qq + 1) * R],
                                  in_=xt8_d[:, :, :, qq * R:(qq + 1) * R])
            wv8 = sp.tile([128, 4, 2, D], f8, tag="wv8")
            nc.sync.dma_start(out=wv8, in_=wv8_d)
            wd8 = sp.tile([64, 8, 2, D], f8, tag="wd8")
            nc.sync.dma_start(out=wd8, in_=wd8_d)
            wcb8 = sp.tile([128, 4, 2, H], f8, tag="wcb8")
            nc.sync.dma_start(out=wcb8, in_=wcb8_d)
            mt_sb = sp.tile([128, 8, H], f32, tag="mt")
            nc.sync.dma_start(out=mt_sb, in_=mt_d)
            gamB = sp.tile([128, D], bf16, tag="gamB")
            nc.sync.dma_start(out=gamB, in_=bcast_row(gam_d[0:1, :], D))
            betB = sp.tile([128, D], bf16, tag="betB")
            nc.sync.dma_start(out=betB, in_=bcast_row(bet_d[0:1, :], D))
            epsT = sp.tile([128, 1], f32, tag="epsT")
            nc.vector.memset(epsT, 1e-5 * 256.0 * 256.0)

            # persistent intermediates
            qT = sp.tile([128, 8, R], bf16, tag="qT")
            qm = sp.tile([128, H, 4, 2, R], f8, tag="qm")
            ctxu = sp.tile([65, H, R], bf16, tag="ctxu")
            ctxn = sp.tile([64, 8, 2, R], f8, tag="ctxn")

            # q projection (own rows = j 0:512), psum = 16*q
            for cb2 in range(8):
                ps = pp.tile([128, R], f32, tag="A", bufs=4, name=f"qps_{cb2}")
                for dg in range(4):
                    nc.tensor.matmul(ps, wq8[:, dg, :, cb2 * 128:(cb2 + 1) * 128],
                                     xt8[:, dg, :, 0:R],
                                     start=(dg == 0), stop=(dg == 3),
                                     perf_mode=DR)
                nc.scalar.copy(out=qT[:, cb2, :], in_=ps)

            # qm for head-group 0 on the scalar engine (early window)
            for h in range(4):
                for g in range(4):
                    for ko in range(2):
                        nc.scalar.mul(out=qm[:, h, g, ko, :],
                                      in_=qT[:, 2 * g + ko, :],
                                      mul=mt_sb[:, 2 * g + ko, h:h + 1])

            for q in range(NQ):
                jq = slice(q * R, (q + 1) * R)

                # k projection for this quarter -> fp8 (16*k)
                kT8 = sp.tile([128, 4, 2, R], f8, tag="kT8", bufs=2,
                              name=f"kT8_{q}")
                for cb2 in range(8):
                    ps = pp.tile([128, R], f32, tag="A", bufs=4,
                                 name=f"kps_{q}_{cb2}")
                    for dg in range(4):
                        nc.tensor.matmul(ps, wk8[:, dg, :, cb2 * 128:(cb2 + 1) * 128],
                                         xt8[:, dg, :, jq],
                                         start=(dg == 0), stop=(dg == 3),
                                         perf_mode=DR)
                    nc.vector.tensor_copy(out=kT8[:, cb2 // 2, cb2 % 2, :], in_=ps)

                # v projection (+ ones column) and content bias
                vA = sp.tile([128, 2, 2, H * 65], f8, tag="vA", bufs=2,
                             name=f"vA_{q}")
                cbT = sp.tile([128, 4, H], f32, tag="cbT", bufs=2, name=f"cbT_{q}")
                for jtp in range(2):
                    for ko in range(2):
                        ev = vA[:, jtp, ko, :].rearrange("p (h u) -> p h u", u=65)
                        nc.vector.memset(ev[:, :, 64:65], 1.0)
                for jt in range(4):
                    jb = slice(q * R + jt * 128, q * R + jt * 128 + 128)
                    psv = [pp.tile([128, R], f32, tag="A", bufs=4,
                                   name=f"vps_{q}_{jt}_{half}")
                           for half in range(2)]
                    pscb = pp.tile([128, H], f32, tag="C", bufs=1,
                                   name=f"cbps_{q}_{jt}")
                    for dg in range(4):
                        for half in range(2):
                            nc.tensor.matmul(psv[half], xt8[:, dg, :, jb],
                                             wv8[:, dg, :, half * 512:(half + 1) * 512],
                                             start=(dg == 0), stop=(dg == 3),
                                             perf_mode=DR)
                        nc.tensor.matmul(pscb, xt8[:, dg, :, jb], wcb8[:, dg, :, :],
                                         start=(dg == 0), stop=(dg == 3),
                                         perf_mode=DR)
                    ev = vA[:, jt // 2, jt % 2, :].rearrange("p (h u) -> p h u", u=65)
                    for half in range(2):
                        nc.vector.tensor_copy(
                            out=ev[:, half * 8:(half + 1) * 8, 0:64],
                            in_=psv[half].rearrange("p (h u) -> p h u", u=64))
                    nc.vector.tensor_scalar(out=cbT[:, jt, :], in0=pscb,
                                            scalar1=1.0 / 16.0, scalar2=None,
                                            op0=ALU.mult)

                if q == 0:
                    # remaining qm (DVE), after quarter-0 casts in program order
                    for h in range(4, H):
                        for g in range(4):
                            for ko in range(2):
                                nc.vector.tensor_scalar_mul(
                                    qm[:, h, g, ko, :], qT[:, 2 * g + ko, :],
                                    mt_sb[:, 2 * g + ko, h:h + 1])

                # attention: head groups of 4
                for hg in range(4):
                    prs = []
                    for hh in range(4):
                        pr = sp.tile([128, 2, 2, R], f8, tag="pr", bufs=4,
                                     name=f"pr_{q}_{hg}_{hh}")
                        prs.append(pr)
                    for jt in range(4):
                        pss = []
                        for hh in range(4):
                            pss.append(pp.tile([128, R], f32, tag="A", bufs=4,
                                               name=f"sps_{q}_{hg}_{jt}_{hh}"))
                        for g in range(4):
                            for hh in range(4):
                                h = hg * 4 + hh
                                nc.tensor.matmul(
                                    pss[hh], kT8[:, g, :, jt * 128:(jt + 1) * 128],
                                    qm[:, h, g, :, :],
                                    start=(g == 0), stop=(g == 3), perf_mode=DR)
                        for hh in range(4):
                            h = hg * 4 + hh
                            nc.scalar.activation(
                                out=prs[hh][:, jt // 2, jt % 2, :], in_=pss[hh],
                                func=AF.Exp, bias=cbT[:, jt, h:h + 1],
                                scale=1.0 / 1024.0)
                    h0 = hg * 4
                    cps4 = pp.tile([65, 4, R], f32, tag="C", bufs=1,
                                   name=f"cps4_{q}_{hg}")
                    for hh in range(4):
                        h = h0 + hh
                        for jtp in range(2):
                            nc.tensor.matmul(cps4[:, hh, :],
                                             vA[:, jtp, :, h * 65:h * 65 + 65],
                                             prs[hh][:, jtp, :, :],
                                             start=(jtp == 0), stop=(jtp == 1),
                                             perf_mode=DR)
                    if q == 0:
                        nc.vector.tensor_copy(out=ctxu[:, h0:h0 + 4, :], in_=cps4)
                    else:
                        nc.vector.tensor_tensor(out=ctxu[:, h0:h0 + 4, :], in0=cps4,
                                                in1=ctxu[:, h0:h0 + 4, :],
                                                op=ALU.add)

                    if q == NQ - 1:
                        # den -> reciprocal -> normalized fp8 ctx, per head
                        # group, overlapping the remaining score matmuls
                        h0 = hg * 4
                        nc.sync.dma_start(out=den_dram[:, h0:h0 + 4, :],
                                          in_=ctxu[64:65, h0:h0 + 4, :])
                        dl = sp.tile([4, R], bf16, tag="dl", bufs=2,
                                     name=f"dl_{hg}")
                        dsrc = bass.AP(tensor=den_dram.tensor,
                                       offset=den_dram[0:1, h0:h0 + 4, :].offset,
                                       ap=[[R, 4], [1, R]])
                        nc.sync.dma_start(out=dl, in_=dsrc)
                        rec4 = sp.tile([4, R], f32, tag="rec", bufs=2,
                                       name=f"rec_{hg}")
                        nc.vector.reciprocal(out=rec4, in_=dl)
                        nc.sync.dma_start(out=rec_dram[h0:h0 + 4, :], in_=rec4)
                        for hh in range(4):
                            h = h0 + hh
                            rb = sp.tile([64, R], f32, tag="rb", bufs=2,
                                         name=f"rb_{h}")
                            src = bass.AP(tensor=rec_dram.tensor,
                                          offset=rec_dram[h:h + 1, :].offset,
                                          ap=[[0, 64], [1, R]])
                            nc.sync.dma_start(out=rb, in_=src)
                            nc.vector.tensor_tensor(
                                out=ctxn[:, h // 2, h % 2, :],
                                in0=ctxu[0:64, h, :], in1=rb, op=ALU.mult)

            # dense (psum = 256*(ctx@Wd)) + residual (xr2 = 256*(x+bd')) + LN
            for ic in range(4):
                res = sp.tile([128, D], f32, tag="res", bufs=1, name=f"res_{ic}")
                xrc = sp.tile([128, D], f32, tag="xrc", bufs=1, name=f"xrc_{ic}")
                nc.sync.dma_start(out=xrc, in_=xr2_d[ic * 128:(ic + 1) * 128, :])
                for oh in range(2):
                    ps = pp.tile([128, 512], f32, tag="A", bufs=4,
                                 name=f"dps_{ic}_{oh}")
                    for s in range(8):
                        nc.tensor.matmul(ps, ctxn[:, s, :, ic * 128:(ic + 1) * 128],
                                         wd8[:, s, :, oh * 512:(oh + 1) * 512],
                                         start=(s == 0), stop=(s == 7),
                                         perf_mode=DR)
                    nc.vector.tensor_tensor(
                        out=res[:, oh * 512:(oh + 1) * 512], in0=ps,
                        in1=xrc[:, oh * 512:(oh + 1) * 512], op=ALU.add)
                stats = sp.tile([128, 2, nc.vector.BN_STATS_DIM], f32, tag="stats",
                                bufs=2, name=f"stats_{ic}")
                for g in range(2):
                    nc.vector.bn_stats(out=stats[:, g, :],
                                       in_=res[:, g * 512:(g + 1) * 512])
                mv = sp.tile([128, nc.vector.BN_AGGR_DIM], f32, tag="mv", bufs=2,
                             name=f"mv_{ic}")
                nc.vector.bn_aggr(out=mv, in_=stats)
                rstd = sp.tile([128, 1], f32, tag="rstd", bufs=2, name=f"rstd_{ic}")
                nc.scalar.activation(out=rstd, in_=mv[:, 1:2], func=AF.Sqrt,
                                     bias=epsT, scale=1.0)
                nc.vector.reciprocal(out=rstd, in_=rstd)
                lnA = sp.tile([128, D], f32, tag="lnA", bufs=2, name=f"lnA_{ic}")
                nc.vector.tensor_scalar(out=lnA, in0=res, scalar1=mv[:, 0:1],
                                        scalar2=rstd, op0=ALU.subtract,
                                        op1=ALU.mult)
                nc.gpsimd.tensor_tensor(out=res, in0=lnA, in1=gamB, op=ALU.mult)
                nc.gpsimd.tensor_tensor(out=lnA, in0=res, in1=betB, op=ALU.add)
                nc.sync.dma_start(out=out_d[ic * 128:(ic + 1) * 128, :], in_=lnA)

    nc.compile()
    return nc


def _arr8(mat, scale):
    """[Drows, C] f32 -> [128, 4, 2, C] fp8 with rows d = dg*256 + ko*128 + p."""
    import ml_dtypes
    a = (mat * scale).astype(ml_dtypes.float8_e4m3)
    C = a.shape[1]
    return np.ascontiguousarray(a.reshape(4, 2, 128, C).transpose(2, 0, 1, 3))


def _prep_in_maps(inputs):
    import ml_dtypes
    f = np.float32
    x = np.ascontiguousarray(np.asarray(inputs["hidden_states"], f))
    Wq = np.asarray(inputs["Wq"], f)
    Wk = np.asarray(inputs["Wk"], f)
    Wcb = np.asarray(inputs["Wcb"], f)
    Wv = np.asarray(inputs["Wv"], f)
    bv = np.asarray(inputs["bv"], f)
    mixing = np.asarray(inputs["mixing"], f)
    Wd = np.asarray(inputs["Wd"], f)
    bd = np.asarray(inputs["bd"], f)
    gamma = np.asarray(inputs["gamma"], f)
    beta = np.asarray(inputs["beta"], f)

    bde = (bd + Wd @ bv).astype(f)
    mt = np.ascontiguousarray(
        (mixing.T * 0.5).reshape(8, 128, H).transpose(1, 0, 2)).astype(f)
    shared = {
        "wq8": _arr8(Wq.T, 16.0),
        "wk8": _arr8(Wk.T, 16.0),
        "wv8": _arr8(Wv.T, 16.0),
        # dense stationary convention: v = (2s+ko)*64 + u -> [u, s, ko, o]
        "wd8": np.ascontiguousarray(
            (Wd.T * 16.0).astype(ml_dtypes.float8_e4m3)
            .reshape(8, 2, 64, D).transpose(2, 0, 1, 3)),
        "wcb8": _arr8(Wcb.T, 2.0),
        "mt": mt,
        "gamma2": np.ascontiguousarray(gamma[None, :]).astype(ml_dtypes.bfloat16),
        "beta2": np.ascontiguousarray(beta[None, :]).astype(ml_dtypes.bfloat16),
    }
    xb8 = [np.asarray(x[b].T, ml_dtypes.float8_e4m3) for b in range(B)]
    in_maps = []
    for c in range(8):
        b, rb = divmod(c, 4)
        r0 = rb * R
        cols = np.r_[r0:r0 + R, 0:r0, r0 + R:S]
        xp = xb8[b][:, cols]
        xt8 = np.ascontiguousarray(xp.reshape(4, 2, 128, S).transpose(2, 0, 1, 3))
        xr2 = np.ascontiguousarray(256.0 * (x[b, r0:r0 + R] + bde[None, :]))
        in_maps.append({"xt8": xt8, "xr2": xr2, **shared})
    return in_maps


def _gather(results):
    out = np.empty((B, S, D), np.float32)
    for c in range(8):
        b, rb = divmod(c, 4)
        out[b, rb * R:(rb + 1) * R] = results[c]["out"]
    return out


def kernel(**inputs):
    from concourse.bass_utils import run_bass_kernel_spmd

    if "nc" not in _CACHE:
        _CACHE["nc"] = _build()
    nc = _CACHE["nc"]
    in_maps = _prep_in_maps(inputs)
    res = run_bass_kernel_spmd(nc, in_maps, core_ids=list(range(8)))
    return (_gather(res.results),)
